# revision 12
# baseline (speedup 1.0000x reference)
"""Causal self-attention on 8 TRN2 NeuronCores — v4 (paired heads, span-major).

Problem: x[2,2048,1024], wq/wk/wv/wo[1024,1024] (nn.Linear convention,
out = y @ W.T), H=16 heads, D=64, causal softmax, f32 I/O.

Sharding: tensor-parallel over heads x data-parallel over batch.
Core i handles batch b=i//4 and head group g=i%4 (4 heads each);
each core returns an f16 partial output projection and the host sums
the 4 partials per batch in f32.

v4 over v3:
- Scores for a head PAIR co-run in the PE array: head-even (kt/qt
  partitions 0-63) at tile_position (0,0), head-odd (64-127) at
  (64,0).  Two K=64 matmuls in disjoint row halves execute
  concurrently, halving scores PE time.
- Span-major attention loop: per (pair, span of 512 queries), ki
  ascending.  Every PSUM accumulation group starts full-width so no
  has_written memset tricks are needed.
- No mask matmuls: diagonal blocks get a triangular bf16 0/1 tile
  multiplied into pt on DVE after exp.
- One exp call per step covers both heads ([128, 1024] PSUM read),
  fewer + wider ACTIVATEs than v3.
- Rowsum broadcast for free: V tiles carry 64 ones-columns, so the PV
  matmul writes the rowsum replicated across PSUM partitions 64-127.
  Norm = reciprocal_approx_fast + tensor_mul on DVE, no DRAM round
  trip (v3's s_dram bounce is gone).
"""

import sys

for _p in ("/opt/trn_rl_repo", "/root/.axon_site"):
    if _p not in sys.path:
        sys.path.insert(0, _p)

import numpy as np
import ml_dtypes

import concourse.bass as bass
import concourse.mybir as mybir
import concourse.tile as tile
from concourse import bacc
from concourse.bass_utils import run_bass_kernel_spmd

B, T, C, H = 2, 2048, 1024, 16
DH = C // H            # 64 head dim
HG = 4                 # heads per core
GW = HG * DH           # 256 features per head group
NB = T // 128          # 16 key chunks
NS = T // 512          # 4 query spans
KC = C // 128          # 8 contraction chunks over C
SCALE = 1.0 / float(np.sqrt(DH))
N_CORES = 8

F32 = mybir.dt.float32
F16 = mybir.dt.float16
BF16 = mybir.dt.bfloat16
EXP = mybir.ActivationFunctionType.Exp
COPY = mybir.ActivationFunctionType.Copy


def build_nc():
    nc = bacc.Bacc("TRN2", target_bir_lowering=False, debug=False,
                   num_devices=N_CORES)
    # All inputs are HOST-PACKED to the exact SBUF tile layouts so every
    # DMA reads long contiguous runs (2-8 KB per partition row) instead
    # of 256B-1KB strided rows — the unpacked layouts measured ~185 GB/s
    # during the startup-critical window.
    xP = nc.declare_dram_parameter("xP", [128, NS * KC * 512], BF16,
                                   isOutput=False)
    wq0P = nc.declare_dram_parameter("wq0P", [128, KC * 128], BF16,
                                     isOutput=False)
    wq1P = nc.declare_dram_parameter("wq1P", [128, KC * 128], BF16,
                                     isOutput=False)
    wk0P = nc.declare_dram_parameter("wk0P", [128, KC * 128], BF16,
                                     isOutput=False)
    wk1P = nc.declare_dram_parameter("wk1P", [128, KC * 128], BF16,
                                     isOutput=False)
    wvP = nc.declare_dram_parameter("wvP", [128, KC * GW], BF16,
                                    isOutput=False)
    woP = nc.declare_dram_parameter("woP", [128, 2 * C], BF16,
                                    isOutput=False)
    outT = nc.declare_dram_parameter("outT", [C, T], F16, isOutput=True)

    with tile.TileContext(nc) as tc:
        with tc.tile_pool(name="pers", bufs=1) as pers, \
             tc.tile_pool(name="MG", bufs=2, space="PSUM") as MG, \
             tc.tile_pool(name="PV", bufs=1, space="PSUM") as PVP, \
             tc.tile_pool(name="PJ", bufs=2, space="PSUM") as PJ, \
             tc.tile_pool(name="PT", bufs=3) as PT, \
             tc.tile_pool(name="NR", bufs=4) as NR, \
             tc.tile_pool(name="OT", bufs=6) as OT:
            # ---- persistent SBUF; DMAs in strict consumption-priority
            # order, ALL on the sync queue (hardware-dynamic path): the
            # DMA engines drain descriptors roughly in issue order, so
            # emission order = arrival order.  Weights for pair-0 go
            # first (they gate the very first matmul), then x span 0,
            # then the rest interleaved.
            def load_w(dram, nch, ncol, tag):
                t = pers.tile([128, nch * ncol], BF16, tag=tag, name=tag)
                nc.sync.dma_start(out=t, in_=dram[:, :])
                return [t[:, i * ncol:(i + 1) * ncol] for i in range(nch)]

            xall = pers.tile([128, KC * T], BF16, tag="xall", name="xall")
            xv = xall.rearrange("p (k t) -> p k t", t=T)
            xts = [xall[:, k * T:(k + 1) * T] for k in range(KC)]
            xPv = xP.rearrange("p (s k c) -> p s k c", k=KC, c=512)

            def load_x(ks, s):
                cols = slice(s * 512, (s + 1) * 512)
                nc.sync.dma_start(out=xv[:, ks, cols], in_=xPv[:, s, ks, :])

            wk_p = [None, None]
            wq_p = [None, None]
            wk_p[0] = load_w(wk0P, KC, 128, "wk0")
            load_x(slice(0, 4), 0)
            wq_p[0] = load_w(wq0P, KC, 128, "wq0")
            load_x(slice(4, 8), 0)
            wv_t = load_w(wvP, KC, GW, "wvall")
            load_x(slice(0, 8), 1)
            wk_p[1] = load_w(wk1P, KC, 128, "wk1")
            wq_p[1] = load_w(wq1P, KC, 128, "wq1")
            load_x(slice(0, 8), 2)
            wo_t = load_w(woP, 2, C, "woall")
            load_x(slice(0, 8), 3)

            qts = [pers.tile([128, T], BF16, tag=f"qT{m}", name=f"qT{m}")
                   for m in range(2)]
            kts = [pers.tile([128, T], BF16, tag=f"kT{m}", name=f"kT{m}")
                   for m in range(2)]
            yts = [pers.tile([128, T], BF16, tag=f"yT{m}", name=f"yT{m}")
                   for m in range(2)]
            # V tiles: per head 128 cols = 64 V dims + 64 ones.  The ones
            # columns make the PV matmul write the rowsum replicated on
            # PSUM partitions 64..127 (free cross-partition broadcast).
            vts = [pers.tile([128, HG * 128], BF16, tag=f"V{tb}",
                             name=f"V{tb}") for tb in range(NB)]
            for tb in range(NB):
                nc.vector.memset(
                    vts[tb].rearrange("p (h c) -> p h c", c=128)[:, :, 64:128],
                    1.0)

            # triangular keep-mask: tri[p, c] = 1 where c >= p else 0
            tri = pers.tile([128, 128], BF16, tag="tri", name="tri")
            nc.gpsimd.memset(tri, 1.0)
            nc.gpsimd.affine_select(
                out=tri, in_=tri, compare_op=mybir.AluOpType.is_ge,
                fill=0.0, base=0, pattern=[[1, 128]], channel_multiplier=-1)

            # ---- emission helpers ----
            def qk_group(which, m, s):
                wt = wk_p[m] if which == "k" else wq_p[m]
                dst = kts[m] if which == "k" else qts[m]
                ps = PJ.tile([128, 512], F32, tag="pj", name="pj")
                for k in range(KC):
                    nc.tensor.matmul(
                        ps, wt[k], xts[k][:, s * 512:(s + 1) * 512],
                        start=(k == 0), stop=(k == KC - 1))
                nc.vector.tensor_copy(
                    out=dst[:, s * 512:(s + 1) * 512], in_=ps)

            def v_group(tb):
                vps = PJ.tile([128, 512], F32, tag="pj", name="pj")
                for k in range(KC):
                    nc.tensor.matmul(
                        vps[:, 0:GW], xts[k][:, tb * 128:(tb + 1) * 128],
                        wv_t[k], start=(k == 0), stop=(k == KC - 1))
                nc.vector.tensor_copy(
                    out=vts[tb].rearrange("p (h c) -> p h c", c=128)[:, :, 0:64],
                    in_=vps.rearrange("p (h c) -> p h c", c=64)[:, 0:4, :])

            def op_group(m, s, use_scalar=False):
                op = PJ.tile([128, 512], F32, tag="pj", name="pj")
                for j in range(2):
                    nc.tensor.matmul(
                        op, wo_t[j][:, m * 128:(m + 1) * 128],
                        yts[j][:, s * 512:(s + 1) * 512],
                        start=(j == 0), stop=(j == 1))
                ot = OT.tile([128, 512], F16, tag="ot", name="ot")
                if use_scalar:
                    nc.scalar.activation(out=ot, in_=op, func=COPY)
                else:
                    nc.vector.tensor_copy(out=ot, in_=op)
                nc.sync.dma_start(
                    out=outT[m * 128:(m + 1) * 128, s * 512:(s + 1) * 512],
                    in_=ot)

            def norm(P, par, pv, s):
                """yts[P][par*64 : +64, span s] = pv[0:64] / rowsum.

                pv[64:128] holds the rowsum replicated by the V ones
                columns.  HW custom-DVE ops ignore the input AP's base
                partition (reciprocal with in_ at base 64 silently reads
                base 0 — measured), but plain tensor_copy / tensor_mul
                from PSUM at base 64 work (HW-probed).  So: copy-shift
                the rowsums to SBUF base 0, recip there, multiply the y
                half of pv directly."""
                po = par * 64
                sh = NR.tile([64, 512], F32, tag="sh", name="sh")
                nc.vector.tensor_copy(out=sh, in_=pv[64:128, :])
                rb = NR.tile([64, 512], F32, tag="rb", name="rb")
                nc.vector.reciprocal_approx_fast(out=rb, in_=sh)
                nc.vector.tensor_mul(
                    out=yts[P][po:po + 64, s * 512:(s + 1) * 512],
                    in0=pv[0:64, :], in1=rb)

            # ---- attention region for (pair P, span s).
            # ki ascending 0..4s+3; last 4 ki are diagonal blocks.
            # PV trails by one step so exp never gates the PE queue.
            def attn_region(P, s, fillers):
                kt, qt = kts[P], qts[P]
                qlo = s * 512
                nki = 4 * s + 4
                pvE = PVP.tile([128, 512], F32, tag="pvE", name="pvE")
                pvO = PVP.tile([128, 512], F32, tag="pvO", name="pvO")
                pend = None

                hE, hO = 2 * P, 2 * P + 1

                def emit_pv(ki, w0, pt):
                    st, fin = (ki == 0), (ki == nki - 1)
                    nc.tensor.matmul(
                        pvE[:, w0:512],
                        vts[ki][:, hE * 128:hE * 128 + 128],
                        pt[:, w0:512], start=st, stop=fin)
                    if fin:
                        norm(P, 0, pvE, s)
                    nc.tensor.matmul(
                        pvO[:, w0:512],
                        vts[ki][:, hO * 128:hO * 128 + 128],
                        pt[:, 512 + w0:1024], start=st, stop=fin)
                    if fin:
                        norm(P, 1, pvO, s)

                for ki in range(nki):
                    diag = ki >= 4 * s
                    w0 = 128 * (ki - 4 * s) if diag else 0
                    mg = MG.tile([128, 1024], F32, tag="mg", name="mg")
                    nc.tensor.matmul(
                        mg[:, w0:512],
                        kt[0:64, ki * 128:(ki + 1) * 128],
                        qt[0:64, qlo + w0:qlo + 512],
                        start=True, stop=True)
                    # head-odd writes full width even on diagonal steps so
                    # the pair-wide exp below reads only freshly-written
                    # PSUM; the invalid prefix cols are never read by PV.
                    nc.tensor.matmul(
                        mg[:, 512:1024],
                        kt[64:128, ki * 128:(ki + 1) * 128],
                        qt[64:128, qlo:qlo + 512],
                        start=True, stop=True)
                    pt = PT.tile([128, 1024], BF16, tag="pt", name="pt")
                    nc.scalar.activation(out=pt[:, w0:1024], in_=mg[:, w0:1024],
                                         func=EXP, scale=SCALE)
                    if diag:
                        nc.vector.tensor_mul(
                            out=pt[:, w0:w0 + 128],
                            in0=pt[:, w0:w0 + 128], in1=tri)
                        nc.vector.tensor_mul(
                            out=pt[:, 512 + w0:512 + w0 + 128],
                            in0=pt[:, 512 + w0:512 + w0 + 128], in1=tri)
                    if fillers:
                        f = fillers.pop(0)
                        if f is not None:
                            f()
                    if pend is not None:
                        emit_pv(*pend)
                    pend = (ki, w0, pt)
                emit_pv(*pend)

                # ---- head of the vts ones-slice is static; the V data
                # columns get filled by v_group fillers.

            # ---- schedule ----
            def KQ(which, m, s):
                return lambda: qk_group(which, m, s)

            def VG(tb):
                return lambda: v_group(tb)

            def OPG(m, s):
                return lambda: op_group(m, s, use_scalar=(m % 2 == 1))

            # startup: pair-0 span-0 projections + V0-3 directly
            qk_group("k", 0, 0)
            qk_group("q", 0, 0)
            for tb in range(4):
                v_group(tb)

            fill = {
                (0, 0): [KQ("k", 1, 0), KQ("q", 1, 0)],
                (1, 0): [KQ("k", 0, 1), KQ("q", 0, 1), VG(4), VG(5)],
                (0, 1): [VG(6), VG(7), KQ("k", 1, 1), KQ("q", 1, 1),
                         OPG(0, 0), OPG(1, 0), OPG(2, 0), OPG(3, 0)],
                (1, 1): [KQ("k", 0, 2), KQ("q", 0, 2), VG(8), VG(9),
                         OPG(4, 0), OPG(5, 0), OPG(6, 0), OPG(7, 0)],
                (0, 2): [VG(10), VG(11), KQ("k", 1, 2), KQ("q", 1, 2)] +
                        [OPG(m, 1) for m in range(8)],
                (1, 2): [KQ("k", 0, 3), KQ("q", 0, 3),
                         VG(12), VG(13), VG(14), VG(15)],
                (0, 3): [KQ("k", 1, 3), KQ("q", 1, 3)] +
                        [OPG(m, 2) for m in range(4)],
                # (1, 3): park the remaining span-2 out-projections at the
                # END of the region so they execute during the final norm
                # chain, keeping the PE warm into the tail.
                (1, 3): [None] * 12 + [OPG(m, 2) for m in range(4, 8)],
            }
            for s in range(NS):
                for P in range(2):
                    attn_region(P, s, fill[(P, s)])

            # tail: final span out-projections
            for m in range(8):
                op_group(m, 3, use_scalar=(m % 2 == 1))
    nc.compile()
    return nc


_NC_CACHE = None


def _get_nc():
    global _NC_CACHE
    if _NC_CACHE is None:
        _NC_CACHE = build_nc()
    return _NC_CACHE


def make_in_maps(x, wq, wk, wv, wo):
    BF = ml_dtypes.bfloat16
    x = np.asarray(x, dtype=np.float32)
    wq = np.asarray(wq, dtype=np.float32)
    wk = np.asarray(wk, dtype=np.float32)
    wv = np.asarray(wv, dtype=np.float32)
    wo = np.asarray(wo, dtype=np.float32)
    def pack_w(wT, ncol):
        # [C_contraction, ncols] -> [128, KC*ncols] tile layout:
        # row p = concat over k of wT[k*128+p, :]
        nch = wT.shape[0] // 128
        return np.ascontiguousarray(
            wT.reshape(nch, 128, ncol).transpose(1, 0, 2).reshape(128, -1)
        ).astype(BF)

    in_maps = []
    for core in range(N_CORES):
        b, g = core // HG, core % HG
        rows = slice(g * GW, (g + 1) * GW)
        xT = x[b].T  # [C, T]
        # [128, s, k, c]: xP[p, s, k, c] = xT[k*128+p, s*512+c]
        xPk = xT.reshape(KC, 128, NS, 512).transpose(1, 2, 0, 3)
        wqT = wq[rows, :].T
        wkT = wk[rows, :].T
        in_maps.append({
            "xP": np.ascontiguousarray(xPk.reshape(128, -1)).astype(BF),
            "wq0P": pack_w(wqT[:, 0:128], 128),
            "wq1P": pack_w(wqT[:, 128:256], 128),
            "wk0P": pack_w(wkT[:, 0:128], 128),
            "wk1P": pack_w(wkT[:, 128:256], 128),
            "wvP": pack_w(wv[rows, :].T, GW),
            "woP": pack_w(wo[:, rows].T, C),
        })
    return in_maps


def run(x, wq, wk, wv, wo, trace=False, tmpdir=None):
    nc = _get_nc()
    in_maps = make_in_maps(x, wq, wk, wv, wo)
    res = run_bass_kernel_spmd(nc, in_maps, core_ids=list(range(N_CORES)),
                               trace=trace, tmpdir=tmpdir)
    out = np.zeros((B, T, C), dtype=np.float32)
    for core in range(N_CORES):
        out[core // HG] += res.results[core]["outT"].T.astype(np.float32)
    return out, res


def kernel(x, wq, wk, wv, wo):
    out, _ = run(x, wq, wk, wv, wo)
    return out


# revision 17
# speedup vs baseline: 1.0170x; 1.0170x over previous
"""Causal self-attention on 8 TRN2 NeuronCores — v4 (paired heads, span-major).

Problem: x[2,2048,1024], wq/wk/wv/wo[1024,1024] (nn.Linear convention,
out = y @ W.T), H=16 heads, D=64, causal softmax, f32 I/O.

Sharding: tensor-parallel over heads x data-parallel over batch.
Core i handles batch b=i//4 and head group g=i%4 (4 heads each);
each core returns an f16 partial output projection and the host sums
the 4 partials per batch in f32.

v4 over v3:
- Scores for a head PAIR co-run in the PE array: head-even (kt/qt
  partitions 0-63) at tile_position (0,0), head-odd (64-127) at
  (64,0).  Two K=64 matmuls in disjoint row halves execute
  concurrently, halving scores PE time.
- Span-major attention loop: per (pair, span of 512 queries), ki
  ascending.  Every PSUM accumulation group starts full-width so no
  has_written memset tricks are needed.
- No mask matmuls: diagonal blocks get a triangular bf16 0/1 tile
  multiplied into pt on DVE after exp.
- One exp call per step covers both heads ([128, 1024] PSUM read),
  fewer + wider ACTIVATEs than v3.
- Rowsum broadcast for free: V tiles carry 64 ones-columns, so the PV
  matmul writes the rowsum replicated across PSUM partitions 64-127.
  Norm = reciprocal_approx_fast + tensor_mul on DVE, no DRAM round
  trip (v3's s_dram bounce is gone).
"""

import sys

for _p in ("/opt/trn_rl_repo", "/root/.axon_site"):
    if _p not in sys.path:
        sys.path.insert(0, _p)

import numpy as np
import ml_dtypes

import concourse.bass as bass
import concourse.mybir as mybir
import concourse.tile as tile
from concourse import bacc
from concourse.bass_utils import run_bass_kernel_spmd

B, T, C, H = 2, 2048, 1024, 16
DH = C // H            # 64 head dim
HG = 4                 # heads per core
GW = HG * DH           # 256 features per head group
NB = T // 128          # 16 key chunks
NS = T // 512          # 4 query spans
KC = C // 128          # 8 contraction chunks over C
SCALE = 1.0 / float(np.sqrt(DH))
N_CORES = 8

F32 = mybir.dt.float32
F16 = mybir.dt.float16
BF16 = mybir.dt.bfloat16
EXP = mybir.ActivationFunctionType.Exp
COPY = mybir.ActivationFunctionType.Copy


def build_nc():
    nc = bacc.Bacc("TRN2", target_bir_lowering=False, debug=False,
                   num_devices=N_CORES)
    # All inputs are HOST-PACKED to the exact SBUF tile layouts so every
    # DMA reads long contiguous runs (2-8 KB per partition row) instead
    # of 256B-1KB strided rows — the unpacked layouts measured ~185 GB/s
    # during the startup-critical window.
    xP = nc.declare_dram_parameter("xP", [128, NS * KC * 512], BF16,
                                   isOutput=False)
    wq0P = nc.declare_dram_parameter("wq0P", [128, KC * 128], BF16,
                                     isOutput=False)
    wq1P = nc.declare_dram_parameter("wq1P", [128, KC * 128], BF16,
                                     isOutput=False)
    wk0P = nc.declare_dram_parameter("wk0P", [128, KC * 128], BF16,
                                     isOutput=False)
    wk1P = nc.declare_dram_parameter("wk1P", [128, KC * 128], BF16,
                                     isOutput=False)
    wvP = nc.declare_dram_parameter("wvP", [128, KC * GW], BF16,
                                    isOutput=False)
    woP = nc.declare_dram_parameter("woP", [128, 2 * C], BF16,
                                    isOutput=False)
    outT = nc.declare_dram_parameter("outT", [C, T], F16, isOutput=True)

    with tile.TileContext(nc) as tc:
        with tc.tile_pool(name="pers", bufs=1) as pers, \
             tc.tile_pool(name="MG", bufs=2, space="PSUM") as MG, \
             tc.tile_pool(name="PV", bufs=1, space="PSUM") as PVP, \
             tc.tile_pool(name="PJ", bufs=2, space="PSUM") as PJ, \
             tc.tile_pool(name="PT", bufs=3) as PT, \
             tc.tile_pool(name="NR", bufs=4) as NR, \
             tc.tile_pool(name="OT", bufs=6) as OT:
            # ---- persistent SBUF; DMAs in strict consumption-priority
            # order, ALL on the sync queue (hardware-dynamic path): the
            # DMA engines drain descriptors roughly in issue order, so
            # emission order = arrival order.  Weights for pair-0 go
            # first (they gate the very first matmul), then x span 0,
            # then the rest interleaved.
            def load_w(dram, nch, ncol, tag):
                t = pers.tile([128, nch * ncol], BF16, tag=tag, name=tag)
                nc.sync.dma_start(out=t, in_=dram[:, :])
                return [t[:, i * ncol:(i + 1) * ncol] for i in range(nch)]

            xall = pers.tile([128, KC * T], BF16, tag="xall", name="xall")
            xv = xall.rearrange("p (k t) -> p k t", t=T)
            xts = [xall[:, k * T:(k + 1) * T] for k in range(KC)]
            xPv = xP.rearrange("p (s k c) -> p s k c", k=KC, c=512)

            def load_x(ks, s):
                cols = slice(s * 512, (s + 1) * 512)
                nc.sync.dma_start(out=xv[:, ks, cols], in_=xPv[:, s, ks, :])

            wk_p = [None, None]
            wq_p = [None, None]
            wk_p[0] = load_w(wk0P, KC, 128, "wk0")
            load_x(slice(0, 4), 0)
            load_x(slice(4, 8), 0)
            wq_p[0] = load_w(wq0P, KC, 128, "wq0")
            wv_t = load_w(wvP, KC, GW, "wvall")
            load_x(slice(0, 8), 1)
            wk_p[1] = load_w(wk1P, KC, 128, "wk1")
            wq_p[1] = load_w(wq1P, KC, 128, "wq1")
            load_x(slice(0, 8), 2)
            wo_t = load_w(woP, 2, C, "woall")
            load_x(slice(0, 8), 3)

            qts = [pers.tile([128, T], BF16, tag=f"qT{m}", name=f"qT{m}")
                   for m in range(2)]
            kts = [pers.tile([128, T], BF16, tag=f"kT{m}", name=f"kT{m}")
                   for m in range(2)]
            yts = [pers.tile([128, T], BF16, tag=f"yT{m}", name=f"yT{m}")
                   for m in range(2)]
            # V tiles: per head 128 cols = 64 V dims + 64 ones.  The ones
            # columns make the PV matmul write the rowsum replicated on
            # PSUM partitions 64..127 (free cross-partition broadcast).
            vts = [pers.tile([128, HG * 128], BF16, tag=f"V{tb}",
                             name=f"V{tb}") for tb in range(NB)]
            for tb in range(NB):
                nc.vector.memset(
                    vts[tb].rearrange("p (h c) -> p h c", c=128)[:, :, 64:128],
                    1.0)

            # triangular keep-mask: tri[p, c] = 1 where c >= p else 0
            tri = pers.tile([128, 128], BF16, tag="tri", name="tri")
            nc.gpsimd.memset(tri, 1.0)
            nc.gpsimd.affine_select(
                out=tri, in_=tri, compare_op=mybir.AluOpType.is_ge,
                fill=0.0, base=0, pattern=[[1, 128]], channel_multiplier=-1)

            # ---- emission helpers ----
            def qk_group(which, m, s):
                wt = wk_p[m] if which == "k" else wq_p[m]
                dst = kts[m] if which == "k" else qts[m]
                ps = PJ.tile([128, 512], F32, tag="pj", name="pj")
                for k in range(KC):
                    nc.tensor.matmul(
                        ps, wt[k], xts[k][:, s * 512:(s + 1) * 512],
                        start=(k == 0), stop=(k == KC - 1))
                nc.vector.tensor_copy(
                    out=dst[:, s * 512:(s + 1) * 512], in_=ps)

            def v_group(tb):
                vps = PJ.tile([128, 512], F32, tag="pj", name="pj")
                for k in range(KC):
                    nc.tensor.matmul(
                        vps[:, 0:GW], xts[k][:, tb * 128:(tb + 1) * 128],
                        wv_t[k], start=(k == 0), stop=(k == KC - 1))
                nc.vector.tensor_copy(
                    out=vts[tb].rearrange("p (h c) -> p h c", c=128)[:, :, 0:64],
                    in_=vps.rearrange("p (h c) -> p h c", c=64)[:, 0:4, :])

            def op_group(m, s, use_scalar=False):
                op = PJ.tile([128, 512], F32, tag="pj", name="pj")
                for j in range(2):
                    nc.tensor.matmul(
                        op, wo_t[j][:, m * 128:(m + 1) * 128],
                        yts[j][:, s * 512:(s + 1) * 512],
                        start=(j == 0), stop=(j == 1))
                ot = OT.tile([128, 512], F16, tag="ot", name="ot")
                if use_scalar:
                    nc.scalar.activation(out=ot, in_=op, func=COPY)
                else:
                    nc.vector.tensor_copy(out=ot, in_=op)
                nc.sync.dma_start(
                    out=outT[m * 128:(m + 1) * 128, s * 512:(s + 1) * 512],
                    in_=ot)

            def norm(P, par, pv, s):
                """yts[P][par*64 : +64, span s] = pv[0:64] / rowsum.

                pv[64:128] holds the rowsum replicated by the V ones
                columns.  HW custom-DVE ops ignore the input AP's base
                partition (reciprocal with in_ at base 64 silently reads
                base 0 — measured), but plain tensor_copy from PSUM at
                base 64 works (HW-probed).  Both pv halves are copied
                out first so the PSUM bank frees early (pv is single
                buffered; the next span's first PV must not wait for the
                whole norm chain)."""
                po = par * 64
                yv = NR.tile([64, 512], F32, tag="yv", name="yv")
                nc.vector.tensor_copy(out=yv, in_=pv[0:64, :])
                sh = NR.tile([64, 512], F32, tag="sh", name="sh")
                nc.vector.tensor_copy(out=sh, in_=pv[64:128, :])
                rb = NR.tile([64, 512], F32, tag="rb", name="rb")
                nc.vector.reciprocal_approx_fast(out=rb, in_=sh)
                nc.vector.tensor_mul(
                    out=yts[P][po:po + 64, s * 512:(s + 1) * 512],
                    in0=yv, in1=rb)

            # ---- attention region for (pair P, span s).
            # ki ascending 0..4s+3; last 4 ki are diagonal blocks.
            # PV trails by one step so exp never gates the PE queue.
            def attn_region(P, s, fillers):
                kt, qt = kts[P], qts[P]
                qlo = s * 512
                nki = 4 * s + 4
                pvE = PVP.tile([128, 512], F32, tag="pvE", name="pvE")
                pvO = PVP.tile([128, 512], F32, tag="pvO", name="pvO")
                pend = None

                hE, hO = 2 * P, 2 * P + 1

                def emit_pv(ki, w0, pt):
                    st, fin = (ki == 0), (ki == nki - 1)
                    nc.tensor.matmul(
                        pvE[:, w0:512],
                        vts[ki][:, hE * 128:hE * 128 + 128],
                        pt[:, w0:512], start=st, stop=fin)
                    if fin:
                        norm(P, 0, pvE, s)
                    nc.tensor.matmul(
                        pvO[:, w0:512],
                        vts[ki][:, hO * 128:hO * 128 + 128],
                        pt[:, 512 + w0:1024], start=st, stop=fin)
                    if fin:
                        norm(P, 1, pvO, s)

                for ki in range(nki):
                    diag = ki >= 4 * s
                    w0 = 128 * (ki - 4 * s) if diag else 0
                    mg = MG.tile([128, 1024], F32, tag="mg", name="mg")
                    nc.tensor.matmul(
                        mg[:, w0:512],
                        kt[0:64, ki * 128:(ki + 1) * 128],
                        qt[0:64, qlo + w0:qlo + 512],
                        start=True, stop=True)
                    # head-odd writes full width even on diagonal steps so
                    # the pair-wide exp below reads only freshly-written
                    # PSUM; the invalid prefix cols are never read by PV.
                    nc.tensor.matmul(
                        mg[:, 512:1024],
                        kt[64:128, ki * 128:(ki + 1) * 128],
                        qt[64:128, qlo:qlo + 512],
                        start=True, stop=True)
                    pt = PT.tile([128, 1024], BF16, tag="pt", name="pt")
                    nc.scalar.activation(out=pt[:, w0:1024], in_=mg[:, w0:1024],
                                         func=EXP, scale=SCALE)
                    if diag:
                        # zero the below-diagonal half of the diag block
                        # in place on GpSimd (idle engine; keeps the DVE
                        # queue clear for the norm chains)
                        for base in (w0, 512 + w0):
                            nc.gpsimd.affine_select(
                                out=pt[:, base:base + 128],
                                in_=pt[:, base:base + 128],
                                compare_op=mybir.AluOpType.is_ge,
                                fill=0.0, base=0, pattern=[[1, 128]],
                                channel_multiplier=-1)
                    if fillers:
                        f = fillers.pop(0)
                        if f is not None:
                            f()
                    if pend is not None:
                        emit_pv(*pend)
                    pend = (ki, w0, pt)
                emit_pv(*pend)

                # ---- head of the vts ones-slice is static; the V data
                # columns get filled by v_group fillers.

            # ---- schedule ----
            def KQ(which, m, s):
                return lambda: qk_group(which, m, s)

            def VG(tb):
                return lambda: v_group(tb)

            # mid-region out-proj copies stay off ScalarE so they never
            # delay the exp cadence; the tail (s=3) runs on ScalarE,
            # which is idle once the last exp retires.
            def OPG(m, s):
                return lambda: op_group(m, s, use_scalar=False)

            # startup: pair-0 span-0 projections + V0-3 directly
            qk_group("k", 0, 0)
            qk_group("q", 0, 0)
            for tb in range(4):
                v_group(tb)

            fill = {
                (0, 0): [KQ("k", 1, 0), KQ("q", 1, 0)],
                (1, 0): [KQ("k", 0, 1), KQ("q", 0, 1), VG(4), VG(5)],
                (0, 1): [VG(6), VG(7), KQ("k", 1, 1), KQ("q", 1, 1),
                         OPG(0, 0), OPG(1, 0), OPG(2, 0), OPG(3, 0)],
                (1, 1): [KQ("k", 0, 2), KQ("q", 0, 2), VG(8), VG(9),
                         OPG(4, 0), OPG(5, 0), OPG(6, 0), OPG(7, 0)],
                (0, 2): [VG(10), VG(11), KQ("k", 1, 2), KQ("q", 1, 2)] +
                        [OPG(m, 1) for m in range(8)],
                (1, 2): [KQ("k", 0, 3), KQ("q", 0, 3),
                         VG(12), VG(13), VG(14), VG(15)],
                (0, 3): [KQ("k", 1, 3), KQ("q", 1, 3)] +
                        [OPG(m, 2) for m in range(4)],
                # (1, 3): park the remaining span-2 out-projections at the
                # END of the region so they execute during the final norm
                # chain, keeping the PE warm into the tail.
                (1, 3): [None] * 12 + [OPG(m, 2) for m in range(4, 8)],
            }
            for s in range(NS):
                for P in range(2):
                    attn_region(P, s, fill[(P, s)])

            # tail: final span out-projections
            for m in range(8):
                op_group(m, 3, use_scalar=True)
    nc.compile()
    return nc


_NC_CACHE = None


def _get_nc():
    global _NC_CACHE
    if _NC_CACHE is None:
        _NC_CACHE = build_nc()
    return _NC_CACHE


def make_in_maps(x, wq, wk, wv, wo):
    BF = ml_dtypes.bfloat16
    x = np.asarray(x, dtype=np.float32)
    wq = np.asarray(wq, dtype=np.float32)
    wk = np.asarray(wk, dtype=np.float32)
    wv = np.asarray(wv, dtype=np.float32)
    wo = np.asarray(wo, dtype=np.float32)
    def pack_w(wT, ncol):
        # [C_contraction, ncols] -> [128, KC*ncols] tile layout:
        # row p = concat over k of wT[k*128+p, :]
        nch = wT.shape[0] // 128
        return np.ascontiguousarray(
            wT.reshape(nch, 128, ncol).transpose(1, 0, 2).reshape(128, -1)
        ).astype(BF)

    in_maps = []
    for core in range(N_CORES):
        b, g = core // HG, core % HG
        rows = slice(g * GW, (g + 1) * GW)
        xT = x[b].T  # [C, T]
        # [128, s, k, c]: xP[p, s, k, c] = xT[k*128+p, s*512+c]
        xPk = xT.reshape(KC, 128, NS, 512).transpose(1, 2, 0, 3)
        wqT = wq[rows, :].T
        wkT = wk[rows, :].T
        in_maps.append({
            "xP": np.ascontiguousarray(xPk.reshape(128, -1)).astype(BF),
            "wq0P": pack_w(wqT[:, 0:128], 128),
            "wq1P": pack_w(wqT[:, 128:256], 128),
            "wk0P": pack_w(wkT[:, 0:128], 128),
            "wk1P": pack_w(wkT[:, 128:256], 128),
            "wvP": pack_w(wv[rows, :].T, GW),
            "woP": pack_w(wo[:, rows].T, C),
        })
    return in_maps


def run(x, wq, wk, wv, wo, trace=False, tmpdir=None):
    nc = _get_nc()
    in_maps = make_in_maps(x, wq, wk, wv, wo)
    res = run_bass_kernel_spmd(nc, in_maps, core_ids=list(range(N_CORES)),
                               trace=trace, tmpdir=tmpdir)
    out = np.zeros((B, T, C), dtype=np.float32)
    for core in range(N_CORES):
        out[core // HG] += res.results[core]["outT"].T.astype(np.float32)
    return out, res


def kernel(x, wq, wk, wv, wo):
    out, _ = run(x, wq, wk, wv, wo)
    return out


# revision 22
# speedup vs baseline: 1.0277x; 1.0105x over previous
"""Causal self-attention on 8 TRN2 NeuronCores — v4 (paired heads, span-major).

Problem: x[2,2048,1024], wq/wk/wv/wo[1024,1024] (nn.Linear convention,
out = y @ W.T), H=16 heads, D=64, causal softmax, f32 I/O.

Sharding: tensor-parallel over heads x data-parallel over batch.
Core i handles batch b=i//4 and head group g=i%4 (4 heads each);
each core returns an f16 partial output projection and the host sums
the 4 partials per batch in f32.

v4 over v3:
- Scores for a head PAIR co-run in the PE array: head-even (kt/qt
  partitions 0-63) at tile_position (0,0), head-odd (64-127) at
  (64,0).  Two K=64 matmuls in disjoint row halves execute
  concurrently, halving scores PE time.
- Span-major attention loop: per (pair, span of 512 queries), ki
  ascending.  Every PSUM accumulation group starts full-width so no
  has_written memset tricks are needed.
- No mask matmuls: diagonal blocks get a triangular bf16 0/1 tile
  multiplied into pt on DVE after exp.
- One exp call per step covers both heads ([128, 1024] PSUM read),
  fewer + wider ACTIVATEs than v3.
- Rowsum broadcast for free: V tiles carry 64 ones-columns, so the PV
  matmul writes the rowsum replicated across PSUM partitions 64-127.
  Norm = reciprocal_approx_fast + tensor_mul on DVE, no DRAM round
  trip (v3's s_dram bounce is gone).
"""

import sys

for _p in ("/opt/trn_rl_repo", "/root/.axon_site"):
    if _p not in sys.path:
        sys.path.insert(0, _p)

import numpy as np
import ml_dtypes

import concourse.bass as bass
import concourse.mybir as mybir
import concourse.tile as tile
from concourse import bacc
from concourse.bass_utils import run_bass_kernel_spmd

B, T, C, H = 2, 2048, 1024, 16
DH = C // H            # 64 head dim
HG = 4                 # heads per core
GW = HG * DH           # 256 features per head group
NB = T // 128          # 16 key chunks
NS = T // 512          # 4 query spans
KC = C // 128          # 8 contraction chunks over C
SCALE = 1.0 / float(np.sqrt(DH))
N_CORES = 8

F32 = mybir.dt.float32
F16 = mybir.dt.float16
BF16 = mybir.dt.bfloat16
EXP = mybir.ActivationFunctionType.Exp
COPY = mybir.ActivationFunctionType.Copy


def build_nc():
    nc = bacc.Bacc("TRN2", target_bir_lowering=False, debug=False,
                   num_devices=N_CORES)
    # All inputs are HOST-PACKED to the exact SBUF tile layouts so every
    # DMA reads long contiguous runs (2-8 KB per partition row) instead
    # of 256B-1KB strided rows — the unpacked layouts measured ~185 GB/s
    # during the startup-critical window.
    xP = nc.declare_dram_parameter("xP", [128, NS * KC * 512], BF16,
                                   isOutput=False)
    wq0P = nc.declare_dram_parameter("wq0P", [128, KC * 128], BF16,
                                     isOutput=False)
    wq1P = nc.declare_dram_parameter("wq1P", [128, KC * 128], BF16,
                                     isOutput=False)
    wk0P = nc.declare_dram_parameter("wk0P", [128, KC * 128], BF16,
                                     isOutput=False)
    wk1P = nc.declare_dram_parameter("wk1P", [128, KC * 128], BF16,
                                     isOutput=False)
    wvP = nc.declare_dram_parameter("wvP", [128, KC * GW], BF16,
                                    isOutput=False)
    woP = nc.declare_dram_parameter("woP", [128, 2 * C], BF16,
                                    isOutput=False)
    outT = nc.declare_dram_parameter("outT", [C, T], F16, isOutput=True)

    with tile.TileContext(nc) as tc:
        with tc.tile_pool(name="pers", bufs=1) as pers, \
             tc.tile_pool(name="MG", bufs=2, space="PSUM") as MG, \
             tc.tile_pool(name="PV", bufs=1, space="PSUM") as PVP, \
             tc.tile_pool(name="PJ", bufs=2, space="PSUM") as PJ, \
             tc.tile_pool(name="PT", bufs=3) as PT, \
             tc.tile_pool(name="NR", bufs=4) as NR, \
             tc.tile_pool(name="OT", bufs=6) as OT:
            # ---- persistent SBUF; DMAs in strict consumption-priority
            # order, ALL on the sync queue (hardware-dynamic path): the
            # DMA engines drain descriptors roughly in issue order, so
            # emission order = arrival order.  Weights for pair-0 go
            # first (they gate the very first matmul), then x span 0,
            # then the rest interleaved.
            def load_w(dram, nch, ncol, tag):
                t = pers.tile([128, nch * ncol], BF16, tag=tag, name=tag)
                nc.sync.dma_start(out=t, in_=dram[:, :])
                return [t[:, i * ncol:(i + 1) * ncol] for i in range(nch)]

            xall = pers.tile([128, KC * T], BF16, tag="xall", name="xall")
            xv = xall.rearrange("p (k t) -> p k t", t=T)
            xts = [xall[:, k * T:(k + 1) * T] for k in range(KC)]
            xPv = xP.rearrange("p (s k c) -> p s k c", k=KC, c=512)

            def load_x(ks, s):
                cols = slice(s * 512, (s + 1) * 512)
                nc.sync.dma_start(out=xv[:, ks, cols], in_=xPv[:, s, ks, :])

            wk_p = [None, None]
            wq_p = [None, None]
            wk_p[0] = load_w(wk0P, KC, 128, "wk0")
            load_x(slice(0, 4), 0)
            load_x(slice(4, 8), 0)
            wq_p[0] = load_w(wq0P, KC, 128, "wq0")
            wv_t = load_w(wvP, KC, GW, "wvall")
            load_x(slice(0, 8), 1)
            wk_p[1] = load_w(wk1P, KC, 128, "wk1")
            wq_p[1] = load_w(wq1P, KC, 128, "wq1")
            load_x(slice(0, 8), 2)
            wo_t = load_w(woP, 2, C, "woall")
            load_x(slice(0, 8), 3)

            qts = [pers.tile([128, T], BF16, tag=f"qT{m}", name=f"qT{m}")
                   for m in range(2)]
            kts = [pers.tile([128, T], BF16, tag=f"kT{m}", name=f"kT{m}")
                   for m in range(2)]
            yts = [pers.tile([128, T], BF16, tag=f"yT{m}", name=f"yT{m}")
                   for m in range(2)]
            # V tiles: per head 128 cols = 64 V dims + 64 ones.  The ones
            # columns make the PV matmul write the rowsum replicated on
            # PSUM partitions 64..127 (free cross-partition broadcast).
            vts = [pers.tile([128, HG * 128], BF16, tag=f"V{tb}",
                             name=f"V{tb}") for tb in range(NB)]
            for tb in range(NB):
                nc.vector.memset(
                    vts[tb].rearrange("p (h c) -> p h c", c=128)[:, :, 64:128],
                    1.0)

            # triangular keep-mask: tri[p, c] = 1 where c >= p else 0
            tri = pers.tile([128, 128], BF16, tag="tri", name="tri")
            nc.gpsimd.memset(tri, 1.0)
            nc.gpsimd.affine_select(
                out=tri, in_=tri, compare_op=mybir.AluOpType.is_ge,
                fill=0.0, base=0, pattern=[[1, 128]], channel_multiplier=-1)

            # ---- emission helpers ----
            def qk_group(which, m, s):
                wt = wk_p[m] if which == "k" else wq_p[m]
                dst = kts[m] if which == "k" else qts[m]
                ps = PJ.tile([128, 512], F32, tag="pj", name="pj")
                for k in range(KC):
                    nc.tensor.matmul(
                        ps, wt[k], xts[k][:, s * 512:(s + 1) * 512],
                        start=(k == 0), stop=(k == KC - 1))
                nc.vector.tensor_copy(
                    out=dst[:, s * 512:(s + 1) * 512], in_=ps)

            def v_group(tb):
                vps = PJ.tile([128, 512], F32, tag="pj", name="pj")
                for k in range(KC):
                    nc.tensor.matmul(
                        vps[:, 0:GW], xts[k][:, tb * 128:(tb + 1) * 128],
                        wv_t[k], start=(k == 0), stop=(k == KC - 1))
                nc.vector.tensor_copy(
                    out=vts[tb].rearrange("p (h c) -> p h c", c=128)[:, :, 0:64],
                    in_=vps.rearrange("p (h c) -> p h c", c=64)[:, 0:4, :])

            def op_group(m, s, use_scalar=False):
                op = PJ.tile([128, 512], F32, tag="pj", name="pj")
                for j in range(2):
                    nc.tensor.matmul(
                        op, wo_t[j][:, m * 128:(m + 1) * 128],
                        yts[j][:, s * 512:(s + 1) * 512],
                        start=(j == 0), stop=(j == 1))
                ot = OT.tile([128, 512], F16, tag="ot", name="ot")
                if use_scalar:
                    nc.scalar.activation(out=ot, in_=op, func=COPY)
                else:
                    nc.vector.tensor_copy(out=ot, in_=op)
                nc.sync.dma_start(
                    out=outT[m * 128:(m + 1) * 128, s * 512:(s + 1) * 512],
                    in_=ot)

            def norm(P, par, pv, s):
                """yts[P][par*64 : +64, span s] = pv[0:64] / rowsum.

                pv[64:128] holds the rowsum replicated by the V ones
                columns.  HW custom-DVE ops ignore the input AP's base
                partition (reciprocal with in_ at base 64 silently reads
                base 0 — measured), but plain tensor_copy from PSUM at
                base 64 works (HW-probed).  Both pv halves are copied
                out first so the PSUM bank frees early (pv is single
                buffered; the next span's first PV must not wait for the
                whole norm chain)."""
                po = par * 64
                yv = NR.tile([64, 512], F32, tag="yv", name="yv")
                nc.vector.tensor_copy(out=yv, in_=pv[0:64, :])
                sh = NR.tile([64, 512], F32, tag="sh", name="sh")
                nc.vector.tensor_copy(out=sh, in_=pv[64:128, :])
                rb = NR.tile([64, 512], F32, tag="rb", name="rb")
                nc.vector.reciprocal_approx_fast(out=rb, in_=sh)
                nc.vector.tensor_mul(
                    out=yts[P][po:po + 64, s * 512:(s + 1) * 512],
                    in0=yv, in1=rb)

            # ---- attention: one flat software-pipelined stream over all
            # (pair P, span s, ki) steps.  Per step: scores pair (co-run
            # row halves) + exp + diag-mask; the PV pair TRAILS by one
            # step, ACROSS region boundaries too — so the last exp of a
            # region overlaps the next region's scores instead of
            # serializing into its own PV (the v4.2 boundary bubble).
            def emit_scores(P, s, ki, w0, diag):
                kt, qt = kts[P], qts[P]
                qlo = s * 512
                mg = MG.tile([128, 1024], F32, tag="mg", name="mg")
                nc.tensor.matmul(
                    mg[:, w0:512],
                    kt[0:64, ki * 128:(ki + 1) * 128],
                    qt[0:64, qlo + w0:qlo + 512],
                    start=True, stop=True)
                # head-odd writes full width even on diagonal steps so
                # the pair-wide exp below reads only freshly-written
                # PSUM; the invalid prefix cols are never read by PV.
                nc.tensor.matmul(
                    mg[:, 512:1024],
                    kt[64:128, ki * 128:(ki + 1) * 128],
                    qt[64:128, qlo:qlo + 512],
                    start=True, stop=True)
                pt = PT.tile([128, 1024], BF16, tag="pt", name="pt")
                nc.scalar.activation(out=pt[:, w0:1024], in_=mg[:, w0:1024],
                                     func=EXP, scale=SCALE)
                if diag:
                    # zero the below-diagonal half of the diag block in
                    # place on GpSimd (idle engine; keeps the DVE queue
                    # clear for the norm chains)
                    for base in (w0, 512 + w0):
                        nc.gpsimd.affine_select(
                            out=pt[:, base:base + 128],
                            in_=pt[:, base:base + 128],
                            compare_op=mybir.AluOpType.is_ge,
                            fill=0.0, base=0, pattern=[[1, 128]],
                            channel_multiplier=-1)
                return pt

            pv_cur = {}  # P -> (pvE, pvO) for the in-flight region

            def emit_pv(P, s, ki, nki, w0, pt):
                st, fin = (ki == 0), (ki == nki - 1)
                if st:
                    pv_cur[P] = (
                        PVP.tile([128, 512], F32, tag="pvE", name="pvE"),
                        PVP.tile([128, 512], F32, tag="pvO", name="pvO"))
                pvE, pvO = pv_cur[P]
                hE, hO = 2 * P, 2 * P + 1
                nc.tensor.matmul(
                    pvE[:, w0:512],
                    vts[ki][:, hE * 128:hE * 128 + 128],
                    pt[:, w0:512], start=st, stop=fin)
                if fin:
                    norm(P, 0, pvE, s)
                nc.tensor.matmul(
                    pvO[:, w0:512],
                    vts[ki][:, hO * 128:hO * 128 + 128],
                    pt[:, 512 + w0:1024], start=st, stop=fin)
                if fin:
                    norm(P, 1, pvO, s)

            # ---- schedule ----
            def KQ(which, m, s):
                return lambda: qk_group(which, m, s)

            def VG(tb):
                return lambda: v_group(tb)

            # mid-region out-proj copies stay off ScalarE so they never
            # delay the exp cadence; the tail (s=3) runs on ScalarE,
            # which is idle once the last exp retires.
            def OPG(m, s):
                return lambda: op_group(m, s, use_scalar=False)

            # HAM warmup: ~32 throwaway matmuls on the tri tile while the
            # first weight/x DMAs stream in.  They put >3.4us of activity
            # into the PE's HAM window so the real projection prologue
            # runs at 2.4 GHz instead of the cold 1.2 GHz default.
            warm = PJ.tile([128, 512], F32, tag="pj", name="pj")
            for _ in range(32):
                nc.tensor.matmul(warm[:, 0:128], tri, tri,
                                 start=True, stop=True)

            # startup: pair-0 span-0 projections + V0-3 directly
            qk_group("k", 0, 0)
            qk_group("q", 0, 0)
            for tb in range(4):
                v_group(tb)

            fill = {
                (0, 0): [KQ("k", 1, 0), KQ("q", 1, 0)],
                (1, 0): [KQ("k", 0, 1), KQ("q", 0, 1), VG(4), VG(5)],
                (0, 1): [VG(6), VG(7), KQ("k", 1, 1), KQ("q", 1, 1),
                         OPG(0, 0), OPG(1, 0), OPG(2, 0), OPG(3, 0)],
                (1, 1): [KQ("k", 0, 2), KQ("q", 0, 2), VG(8), VG(9),
                         OPG(4, 0), OPG(5, 0), OPG(6, 0), OPG(7, 0)],
                (0, 2): [VG(10), VG(11), KQ("k", 1, 2), KQ("q", 1, 2)] +
                        [OPG(m, 1) for m in range(8)],
                (1, 2): [KQ("k", 0, 3), KQ("q", 0, 3),
                         VG(12), VG(13), VG(14), VG(15)],
                (0, 3): [KQ("k", 1, 3), KQ("q", 1, 3)] +
                        [OPG(m, 2) for m in range(4)],
                # (1, 3): park the remaining span-2 out-projections at the
                # END of the region so they execute during the final norm
                # chain, keeping the PE warm into the tail.
                (1, 3): [None] * 12 + [OPG(m, 2) for m in range(4, 8)],
            }

            steps = []
            fillers = []
            for s in range(NS):
                for P in range(2):
                    nki = 4 * s + 4
                    fl = fill[(P, s)]
                    for ki in range(nki):
                        steps.append((P, s, ki, nki))
                        fillers.append(fl[ki] if ki < len(fl) else None)

            pend = None
            for n, (P, s, ki, nki) in enumerate(steps):
                diag = ki >= 4 * s
                w0 = 128 * (ki - 4 * s) if diag else 0
                pt = emit_scores(P, s, ki, w0, diag)
                f = fillers[n]
                if f is not None:
                    f()
                if pend is not None:
                    emit_pv(*pend)
                pend = (P, s, ki, nki, w0, pt)
            emit_pv(*pend)

            # tail: final span out-projections, copies split across both
            # free engines (ScalarE is idle after the last exp, DVE after
            # the last norm)
            for m in range(8):
                op_group(m, 3, use_scalar=(m % 2 == 1))
    nc.compile()
    return nc


_NC_CACHE = None


def _get_nc():
    global _NC_CACHE
    if _NC_CACHE is None:
        _NC_CACHE = build_nc()
    return _NC_CACHE


def make_in_maps(x, wq, wk, wv, wo):
    BF = ml_dtypes.bfloat16
    x = np.asarray(x, dtype=np.float32)
    wq = np.asarray(wq, dtype=np.float32)
    wk = np.asarray(wk, dtype=np.float32)
    wv = np.asarray(wv, dtype=np.float32)
    wo = np.asarray(wo, dtype=np.float32)
    def pack_w(wT, ncol):
        # [C_contraction, ncols] -> [128, KC*ncols] tile layout:
        # row p = concat over k of wT[k*128+p, :]
        nch = wT.shape[0] // 128
        return np.ascontiguousarray(
            wT.reshape(nch, 128, ncol).transpose(1, 0, 2).reshape(128, -1)
        ).astype(BF)

    in_maps = []
    for core in range(N_CORES):
        b, g = core // HG, core % HG
        rows = slice(g * GW, (g + 1) * GW)
        xT = x[b].T  # [C, T]
        # [128, s, k, c]: xP[p, s, k, c] = xT[k*128+p, s*512+c]
        xPk = xT.reshape(KC, 128, NS, 512).transpose(1, 2, 0, 3)
        wqT = wq[rows, :].T
        wkT = wk[rows, :].T
        in_maps.append({
            "xP": np.ascontiguousarray(xPk.reshape(128, -1)).astype(BF),
            "wq0P": pack_w(wqT[:, 0:128], 128),
            "wq1P": pack_w(wqT[:, 128:256], 128),
            "wk0P": pack_w(wkT[:, 0:128], 128),
            "wk1P": pack_w(wkT[:, 128:256], 128),
            "wvP": pack_w(wv[rows, :].T, GW),
            "woP": pack_w(wo[:, rows].T, C),
        })
    return in_maps


def run(x, wq, wk, wv, wo, trace=False, tmpdir=None):
    nc = _get_nc()
    in_maps = make_in_maps(x, wq, wk, wv, wo)
    res = run_bass_kernel_spmd(nc, in_maps, core_ids=list(range(N_CORES)),
                               trace=trace, tmpdir=tmpdir)
    out = np.zeros((B, T, C), dtype=np.float32)
    for core in range(N_CORES):
        out[core // HG] += res.results[core]["outT"].T.astype(np.float32)
    return out, res


def kernel(x, wq, wk, wv, wo):
    out, _ = run(x, wq, wk, wv, wo)
    return out


# revision 25
# speedup vs baseline: 1.0536x; 1.0252x over previous
"""Causal self-attention on 8 TRN2 NeuronCores — v4 (paired heads, span-major).

Problem: x[2,2048,1024], wq/wk/wv/wo[1024,1024] (nn.Linear convention,
out = y @ W.T), H=16 heads, D=64, causal softmax, f32 I/O.

Sharding: tensor-parallel over heads x data-parallel over batch.
Core i handles batch b=i//4 and head group g=i%4 (4 heads each);
each core returns an f16 partial output projection and the host sums
the 4 partials per batch in f32.

v4 over v3:
- Scores for a head PAIR co-run in the PE array: head-even (kt/qt
  partitions 0-63) at tile_position (0,0), head-odd (64-127) at
  (64,0).  Two K=64 matmuls in disjoint row halves execute
  concurrently, halving scores PE time.
- Span-major attention loop: per (pair, span of 512 queries), ki
  ascending.  Every PSUM accumulation group starts full-width so no
  has_written memset tricks are needed.
- No mask matmuls: diagonal blocks get a triangular bf16 0/1 tile
  multiplied into pt on DVE after exp.
- One exp call per step covers both heads ([128, 1024] PSUM read),
  fewer + wider ACTIVATEs than v3.
- Rowsum broadcast for free: V tiles carry 64 ones-columns, so the PV
  matmul writes the rowsum replicated across PSUM partitions 64-127.
  Norm = reciprocal_approx_fast + tensor_mul on DVE, no DRAM round
  trip (v3's s_dram bounce is gone).
"""

import sys

for _p in ("/opt/trn_rl_repo", "/root/.axon_site"):
    if _p not in sys.path:
        sys.path.insert(0, _p)

import numpy as np
import ml_dtypes

import concourse.bass as bass
import concourse.mybir as mybir
import concourse.tile as tile
from concourse import bacc
from concourse.bass_utils import run_bass_kernel_spmd

B, T, C, H = 2, 2048, 1024, 16
DH = C // H            # 64 head dim
HG = 4                 # heads per core
GW = HG * DH           # 256 features per head group
NB = T // 128          # 16 key chunks
NS = T // 512          # 4 query spans
KC = C // 128          # 8 contraction chunks over C
SCALE = 1.0 / float(np.sqrt(DH))
N_CORES = 8

F32 = mybir.dt.float32
F16 = mybir.dt.float16
BF16 = mybir.dt.bfloat16
EXP = mybir.ActivationFunctionType.Exp
COPY = mybir.ActivationFunctionType.Copy


def build_nc():
    nc = bacc.Bacc("TRN2", target_bir_lowering=False, debug=False,
                   num_devices=N_CORES)
    # All inputs are HOST-PACKED to the exact SBUF tile layouts so every
    # DMA reads long contiguous runs (2-8 KB per partition row) instead
    # of 256B-1KB strided rows — the unpacked layouts measured ~185 GB/s
    # during the startup-critical window.
    xP = nc.declare_dram_parameter("xP", [128, NS * KC * 512], BF16,
                                   isOutput=False)
    wq0P = nc.declare_dram_parameter("wq0P", [128, KC * 128], BF16,
                                     isOutput=False)
    wq1P = nc.declare_dram_parameter("wq1P", [128, KC * 128], BF16,
                                     isOutput=False)
    wk0P = nc.declare_dram_parameter("wk0P", [128, KC * 128], BF16,
                                     isOutput=False)
    wk1P = nc.declare_dram_parameter("wk1P", [128, KC * 128], BF16,
                                     isOutput=False)
    wvP = nc.declare_dram_parameter("wvP", [128, KC * GW], BF16,
                                    isOutput=False)
    woP = nc.declare_dram_parameter("woP", [128, 2 * C], BF16,
                                    isOutput=False)
    outT = nc.declare_dram_parameter("outT", [C, T], F16, isOutput=True)

    with tile.TileContext(nc) as tc:
        with tc.tile_pool(name="pers", bufs=1) as pers, \
             tc.tile_pool(name="MG", bufs=2, space="PSUM") as MG, \
             tc.tile_pool(name="PV", bufs=1, space="PSUM") as PVP, \
             tc.tile_pool(name="PJ", bufs=2, space="PSUM") as PJ, \
             tc.tile_pool(name="PT", bufs=3) as PT, \
             tc.tile_pool(name="NR", bufs=4) as NR, \
             tc.tile_pool(name="OT", bufs=6) as OT:
            # ---- persistent SBUF; DMAs in strict consumption-priority
            # order, ALL on the sync queue (hardware-dynamic path): the
            # DMA engines drain descriptors roughly in issue order, so
            # emission order = arrival order.  Weights for pair-0 go
            # first (they gate the very first matmul), then x span 0,
            # then the rest interleaved.
            def load_w(dram, nch, ncol, tag):
                t = pers.tile([128, nch * ncol], BF16, tag=tag, name=tag)
                nc.sync.dma_start(out=t, in_=dram[:, :])
                return [t[:, i * ncol:(i + 1) * ncol] for i in range(nch)]

            xall = pers.tile([128, KC * T], BF16, tag="xall", name="xall")
            xv = xall.rearrange("p (k t) -> p k t", t=T)
            xts = [xall[:, k * T:(k + 1) * T] for k in range(KC)]
            xPv = xP.rearrange("p (s k c) -> p s k c", k=KC, c=512)

            def load_x(ks, s):
                cols = slice(s * 512, (s + 1) * 512)
                nc.sync.dma_start(out=xv[:, ks, cols], in_=xPv[:, s, ks, :])

            wk_p = [None, None]
            wq_p = [None, None]
            wk_p[0] = load_w(wk0P, KC, 128, "wk0")
            load_x(slice(0, 4), 0)
            load_x(slice(4, 8), 0)
            wq_p[0] = load_w(wq0P, KC, 128, "wq0")
            wv_t = load_w(wvP, KC, GW, "wvall")
            load_x(slice(0, 8), 1)
            wk_p[1] = load_w(wk1P, KC, 128, "wk1")
            wq_p[1] = load_w(wq1P, KC, 128, "wq1")
            load_x(slice(0, 8), 2)
            wo_t = load_w(woP, 2, C, "woall")
            load_x(slice(0, 8), 3)

            qts = [pers.tile([128, T], BF16, tag=f"qT{m}", name=f"qT{m}")
                   for m in range(2)]
            kts = [pers.tile([128, T], BF16, tag=f"kT{m}", name=f"kT{m}")
                   for m in range(2)]
            yts = [pers.tile([128, T], BF16, tag=f"yT{m}", name=f"yT{m}")
                   for m in range(2)]
            # V tiles: per head 128 cols = 64 V dims + 64 ones.  The ones
            # columns make the PV matmul write the rowsum replicated on
            # PSUM partitions 64..127 (free cross-partition broadcast).
            vts = [pers.tile([128, HG * 128], BF16, tag=f"V{tb}",
                             name=f"V{tb}") for tb in range(NB)]
            for tb in range(NB):
                nc.vector.memset(
                    vts[tb].rearrange("p (h c) -> p h c", c=128)[:, :, 64:128],
                    1.0)

            # triangular keep-mask: tri[p, c] = 1 where c >= p else 0
            tri = pers.tile([128, 128], BF16, tag="tri", name="tri")
            nc.gpsimd.memset(tri, 1.0)
            nc.gpsimd.affine_select(
                out=tri, in_=tri, compare_op=mybir.AluOpType.is_ge,
                fill=0.0, base=0, pattern=[[1, 128]], channel_multiplier=-1)

            # ---- emission helpers ----
            # Projection fillers are split into ~0.5us sub-fillers (2
            # matmuls each) so the per-step PE load stays close to the
            # exp period — coarse 1.7us fillers made some steps PE-bound
            # while others idled against ScalarE.
            def qk_subs(which, m, s):
                wt = wk_p[m] if which == "k" else wq_p[m]
                dst = kts[m] if which == "k" else qts[m]
                cell = {}

                def sub(i):
                    def f():
                        if i == 0:
                            cell["ps"] = PJ.tile([128, 512], F32, tag="pj",
                                                 name="pj")
                        ps = cell["ps"]
                        for k in (2 * i, 2 * i + 1):
                            nc.tensor.matmul(
                                ps, wt[k],
                                xts[k][:, s * 512:(s + 1) * 512],
                                start=(k == 0), stop=(k == KC - 1))
                        if i == 3:
                            nc.vector.tensor_copy(
                                out=dst[:, s * 512:(s + 1) * 512], in_=ps)
                    return f

                return [sub(i) for i in range(4)]

            def v_subs(tb):
                cell = {}

                def sub(i):
                    def f():
                        if i == 0:
                            cell["ps"] = PJ.tile([128, 512], F32, tag="pj",
                                                 name="pj")
                        vps = cell["ps"]
                        for k in range(4 * i, 4 * i + 4):
                            nc.tensor.matmul(
                                vps[:, 0:GW],
                                xts[k][:, tb * 128:(tb + 1) * 128],
                                wv_t[k], start=(k == 0), stop=(k == KC - 1))
                        if i == 1:
                            nc.vector.tensor_copy(
                                out=vts[tb].rearrange(
                                    "p (h c) -> p h c", c=128)[:, :, 0:64],
                                in_=vps.rearrange(
                                    "p (h c) -> p h c", c=64)[:, 0:4, :])
                    return f

                return [sub(0), sub(1)]

            def qk_group(which, m, s):
                for f in qk_subs(which, m, s):
                    f()

            def op_group(m, s, use_scalar=False):
                op = PJ.tile([128, 512], F32, tag="pj", name="pj")
                for j in range(2):
                    nc.tensor.matmul(
                        op, wo_t[j][:, m * 128:(m + 1) * 128],
                        yts[j][:, s * 512:(s + 1) * 512],
                        start=(j == 0), stop=(j == 1))
                ot = OT.tile([128, 512], F16, tag="ot", name="ot")
                if use_scalar:
                    nc.scalar.activation(out=ot, in_=op, func=COPY)
                else:
                    nc.vector.tensor_copy(out=ot, in_=op)
                nc.sync.dma_start(
                    out=outT[m * 128:(m + 1) * 128, s * 512:(s + 1) * 512],
                    in_=ot)

            def norm(P, par, pv, s):
                """yts[P][par*64 : +64, span s] = pv[0:64] / rowsum.

                pv[64:128] holds the rowsum replicated by the V ones
                columns.  HW custom-DVE ops ignore the input AP's base
                partition (reciprocal with in_ at base 64 silently reads
                base 0 — measured), but plain tensor_copy from PSUM at
                base 64 works (HW-probed).  Both pv halves are copied
                out first so the PSUM bank frees early (pv is single
                buffered; the next span's first PV must not wait for the
                whole norm chain).  For the final region (no next span)
                skip the y copy and multiply straight from PSUM — the
                tail's op matmuls wait on this chain."""
                po = par * 64
                last = (P, s) == (1, NS - 1)
                if not last:
                    yv = NR.tile([64, 512], F32, tag="yv", name="yv")
                    nc.vector.tensor_copy(out=yv, in_=pv[0:64, :])
                sh = NR.tile([64, 512], F32, tag="sh", name="sh")
                nc.vector.tensor_copy(out=sh, in_=pv[64:128, :])
                rb = NR.tile([64, 512], F32, tag="rb", name="rb")
                nc.vector.reciprocal_approx_fast(out=rb, in_=sh)
                nc.vector.tensor_mul(
                    out=yts[P][po:po + 64, s * 512:(s + 1) * 512],
                    in0=(pv[0:64, :] if last else yv), in1=rb)

            # ---- attention: one flat software-pipelined stream over all
            # (pair P, span s, ki) steps.  Per step: scores pair (co-run
            # row halves) + exp + diag-mask; the PV pair TRAILS by one
            # step, ACROSS region boundaries too — so the last exp of a
            # region overlaps the next region's scores instead of
            # serializing into its own PV (the v4.2 boundary bubble).
            def emit_scores(P, s, ki, w0, diag):
                kt, qt = kts[P], qts[P]
                qlo = s * 512
                mg = MG.tile([128, 1024], F32, tag="mg", name="mg")
                nc.tensor.matmul(
                    mg[:, w0:512],
                    kt[0:64, ki * 128:(ki + 1) * 128],
                    qt[0:64, qlo + w0:qlo + 512],
                    start=True, stop=True)
                # head-odd writes full width even on diagonal steps so
                # the pair-wide exp below reads only freshly-written
                # PSUM; the invalid prefix cols are never read by PV.
                nc.tensor.matmul(
                    mg[:, 512:1024],
                    kt[64:128, ki * 128:(ki + 1) * 128],
                    qt[64:128, qlo:qlo + 512],
                    start=True, stop=True)
                pt = PT.tile([128, 1024], BF16, tag="pt", name="pt")
                nc.scalar.activation(out=pt[:, w0:1024], in_=mg[:, w0:1024],
                                     func=EXP, scale=SCALE)
                if diag:
                    # zero the below-diagonal half of the diag block in
                    # place on GpSimd (idle engine; keeps the DVE queue
                    # clear for the norm chains)
                    for base in (w0, 512 + w0):
                        nc.gpsimd.affine_select(
                            out=pt[:, base:base + 128],
                            in_=pt[:, base:base + 128],
                            compare_op=mybir.AluOpType.is_ge,
                            fill=0.0, base=0, pattern=[[1, 128]],
                            channel_multiplier=-1)
                return pt

            pv_cur = {}  # P -> (pvE, pvO) for the in-flight region

            def emit_pv(P, s, ki, nki, w0, pt):
                st, fin = (ki == 0), (ki == nki - 1)
                if st:
                    pv_cur[P] = (
                        PVP.tile([128, 512], F32, tag="pvE", name="pvE"),
                        PVP.tile([128, 512], F32, tag="pvO", name="pvO"))
                pvE, pvO = pv_cur[P]
                hE, hO = 2 * P, 2 * P + 1
                nc.tensor.matmul(
                    pvE[:, w0:512],
                    vts[ki][:, hE * 128:hE * 128 + 128],
                    pt[:, w0:512], start=st, stop=fin)
                if fin:
                    norm(P, 0, pvE, s)
                nc.tensor.matmul(
                    pvO[:, w0:512],
                    vts[ki][:, hO * 128:hO * 128 + 128],
                    pt[:, 512 + w0:1024], start=st, stop=fin)
                if fin:
                    norm(P, 1, pvO, s)

            # ---- schedule ----
            def KQ(which, m, s):
                return lambda: qk_group(which, m, s)

            def VG(tb):
                return lambda: v_group(tb)

            # mid-region out-proj copies stay off ScalarE so they never
            # delay the exp cadence; the tail (s=3) runs on ScalarE,
            # which is idle once the last exp retires.
            def OPG(m, s):
                return lambda: op_group(m, s, use_scalar=False)

            # HAM warmup: throwaway matmuls on the tri tile while the
            # first weight/x DMAs stream in.  They put >3.4us of activity
            # into the PE's HAM window so the real projection prologue
            # runs at 2.4 GHz instead of the cold 1.2 GHz default.
            warm = PJ.tile([128, 512], F32, tag="pj", name="pj")
            for _ in range(48):
                nc.tensor.matmul(warm[:, 0:128], tri, tri,
                                 start=True, stop=True)

            # startup: pair-0 span-0 q/k projections only; V0-3 ride as
            # the first region's fillers.
            qk_group("k", 0, 0)
            qk_group("q", 0, 0)

            fill = {
                (0, 0): v_subs(0) + v_subs(1) + v_subs(2) + v_subs(3) +
                        qk_subs("k", 1, 0) + qk_subs("q", 1, 0),
                (1, 0): qk_subs("k", 0, 1) + qk_subs("q", 0, 1) +
                        v_subs(4) + v_subs(5),
                (0, 1): v_subs(6) + v_subs(7) +
                        qk_subs("k", 1, 1) + qk_subs("q", 1, 1) +
                        [OPG(0, 0), OPG(1, 0)],
                (1, 1): qk_subs("k", 0, 2) + qk_subs("q", 0, 2) +
                        v_subs(8) + v_subs(9) +
                        [OPG(m, 0) for m in range(2, 6)],
                (0, 2): v_subs(10) + v_subs(11) +
                        qk_subs("k", 1, 2) + qk_subs("q", 1, 2) +
                        [OPG(6, 0), OPG(7, 0)] +
                        [OPG(m, 1) for m in range(0, 4)],
                (1, 2): qk_subs("k", 0, 3) + qk_subs("q", 0, 3) +
                        v_subs(12) + v_subs(13) + v_subs(14) + v_subs(15) +
                        [OPG(m, 1) for m in range(4, 8)],
                (0, 3): qk_subs("k", 1, 3) + qk_subs("q", 1, 3) +
                        [OPG(m, 2) for m in range(0, 4)],
                (1, 3): [OPG(m, 2) for m in range(4, 8)],
            }

            steps = []
            fillers = []
            for s in range(NS):
                for P in range(2):
                    nki = 4 * s + 4
                    fl = list(fill[(P, s)])
                    for ki in range(nki):
                        steps.append((P, s, ki, nki))
                        take = -(-len(fl) // (nki - ki))  # ceil pacing
                        fillers.append(fl[:take])
                        fl = fl[take:]

            pend = None
            for n, (P, s, ki, nki) in enumerate(steps):
                diag = ki >= 4 * s
                w0 = 128 * (ki - 4 * s) if diag else 0
                pt = emit_scores(P, s, ki, w0, diag)
                for f in fillers[n]:
                    f()
                if pend is not None:
                    emit_pv(*pend)
                pend = (P, s, ki, nki, w0, pt)
            emit_pv(*pend)

            # tail: final span out-projections, copies split across both
            # free engines (ScalarE is idle after the last exp, DVE after
            # the last norm)
            for m in range(8):
                op_group(m, 3, use_scalar=(m % 2 == 1))
    nc.compile()
    return nc


_NC_CACHE = None


def _get_nc():
    global _NC_CACHE
    if _NC_CACHE is None:
        _NC_CACHE = build_nc()
    return _NC_CACHE


def make_in_maps(x, wq, wk, wv, wo):
    BF = ml_dtypes.bfloat16
    x = np.asarray(x, dtype=np.float32)
    wq = np.asarray(wq, dtype=np.float32)
    wk = np.asarray(wk, dtype=np.float32)
    wv = np.asarray(wv, dtype=np.float32)
    wo = np.asarray(wo, dtype=np.float32)
    def pack_w(wT, ncol):
        # [C_contraction, ncols] -> [128, KC*ncols] tile layout:
        # row p = concat over k of wT[k*128+p, :]
        nch = wT.shape[0] // 128
        return np.ascontiguousarray(
            wT.reshape(nch, 128, ncol).transpose(1, 0, 2).reshape(128, -1)
        ).astype(BF)

    in_maps = []
    for core in range(N_CORES):
        b, g = core // HG, core % HG
        rows = slice(g * GW, (g + 1) * GW)
        xT = x[b].T  # [C, T]
        # [128, s, k, c]: xP[p, s, k, c] = xT[k*128+p, s*512+c]
        xPk = xT.reshape(KC, 128, NS, 512).transpose(1, 2, 0, 3)
        wqT = wq[rows, :].T
        wkT = wk[rows, :].T
        in_maps.append({
            "xP": np.ascontiguousarray(xPk.reshape(128, -1)).astype(BF),
            "wq0P": pack_w(wqT[:, 0:128], 128),
            "wq1P": pack_w(wqT[:, 128:256], 128),
            "wk0P": pack_w(wkT[:, 0:128], 128),
            "wk1P": pack_w(wkT[:, 128:256], 128),
            "wvP": pack_w(wv[rows, :].T, GW),
            "woP": pack_w(wo[:, rows].T, C),
        })
    return in_maps


def run(x, wq, wk, wv, wo, trace=False, tmpdir=None):
    nc = _get_nc()
    in_maps = make_in_maps(x, wq, wk, wv, wo)
    res = run_bass_kernel_spmd(nc, in_maps, core_ids=list(range(N_CORES)),
                               trace=trace, tmpdir=tmpdir)
    out = np.zeros((B, T, C), dtype=np.float32)
    for core in range(N_CORES):
        out[core // HG] += res.results[core]["outT"].T.astype(np.float32)
    return out, res


def kernel(x, wq, wk, wv, wo):
    out, _ = run(x, wq, wk, wv, wo)
    return out
